# revision 51
# baseline (speedup 1.0000x reference)
"""Trainium2 Bass kernel for nn_AttentionHeadCheb (gnn_message_passing).

8-core SPMD, row-sharded (destination-node dim). Each core owns 512 rows of
the [4096, 4096] score/prob matrices for all K=3 hops, streams its mask
shards from HBM once, and computes its 512 output rows. No collectives.

Masks are cast to fp16 {0,1} on the host (the reference's own first op is
`(adj != 0).astype(f32)`), halving HBM traffic vs int32; x is additionally
shipped pre-transposed (layout only) so no PE time is spent transposing it.

Engine cost model measured on HW (per [128, 2048] fp16 op): tensor_scalar
~660ns (4x dual-pump), tensor_tensor ~1140ns (2x), fused scalar_tensor_
tensor ~2350ns (1x -- avoid), PE matmul 56ns/128-col when ramped, gpsimd
~4000ns AND it contends with DVE for SBUF ports (keep it idle). The kernel
balances the score build across PE and DVE accordingly:

  setup:  x_aug  = cast16(x) with a ones column          (for P@x and rowsums)
          xT     = cast16(host-transposed x)             (DMA)
          V      = W_t[k] @ [W_l[k] | W_r[k]].T          (PE, tiny)
          al     = xloc @ V   (left coeffs, local rows)  (PE, tiny)
          arT    = V.T @ xT   (right coeffs, all cols)   (PE)
          AR_c   = broadcast arT rows across partitions  (0-stride DMA)
  main:   per channel c in {amask r=0,1,2 (PE_AL), smask_k}:
            PE_AL channels:  D_c = AR_c * M_c            (DVE TT-mult, 2x)
                             al part rides the PE:       S^T += M_c^T diag(al_c)
                             (stationary = raw mask, moving = diag(al_c);
                              diag built by a tiny DVE tensor_scalar_mul)
            else:            D_c = (AR_c + al_c) * M_c   (DVE TS-add + TT-mult)
          S^T    = sum_c transpose(D_c) (+ diag terms)   (PE transpose-accum,
                   7 matmuls per 128x128 chunk)
          p~T    = exp(S^T - 4), sg = sign(S^T)          (ACT, PSUM -> fp16)
          p~T    = |p~T * sg|                            (valid mask: DVE mult
                   + sign-bit clear; S==0 exactly <=> masked out, as reference)
          G_k    = p~T.T @ [x16 | 1]                     (PE, accumulated in PSUM)
  out:    G'_k   = G_k * (1/Z_k) ;  out^T = sum_k W_t[k].T @ G'_k^T   (PE)
          out    = elu(out^T)                            (ACT exp + DVE select)

Softmax needs no max-subtraction: |scores| <= ~9 for this distribution, the
-4 shift keeps exp() within fp16 range. Rows with no edges give Z=0 -> out 0,
matching the reference. The first block's mask DMAs are issued ahead of the
whole setup stream so the main loop can start as soon as the AR broadcast
tiles land. Steady state is PE-bound at ~99% tensor-engine occupancy.
"""

import threading

import numpy as np

import concourse.bass as bass
import concourse.tile as tile
from concourse import bacc, mybir
from concourse.masks import make_identity

N, DIN, DOUT, K, R = 4096, 256, 256, 3, 3
NCORES = 8
NLOC = N // NCORES  # 512 local rows per core
P = 128
NBLK = NLOC // P  # 4 row blocks
NH = 2  # column halves
H = N // NH  # 2048
NCH = H // P  # 16 column chunks per half
NGRP = 4  # psum S^T groups per half
GJ = NCH // NGRP  # 4 chunks per group
NC = 4  # mask channels: amask r=0..2 + smask
# channels whose al-add rides the PE as a diag matmul, per hop k;
# fractional balance: steady-state PE ran ~82% vs DVE ~74% with 3
PE_AL_K = {0: (0, 1, 2), 1: (0, 1, 2), 2: (0, 1, 2)}
SHIFT = 4.0  # exp shift for fp16 safety

F16 = mybir.dt.float16
F32 = mybir.dt.float32
I32 = mybir.dt.int32
I16 = mybir.dt.int16
AF = mybir.ActivationFunctionType
OP = mybir.AluOpType


def build_program():
    nc = bacc.Bacc(
        "TRN2",
        target_bir_lowering=False,
        debug=False,
        enable_asserts=False,
        num_devices=NCORES,
    )

    # xg/xloc/wt arrive pre-blocked from the host (partition-major) so the
    # staging DMAs read long contiguous per-partition lines
    xg = nc.dram_tensor("xg", (P, N // P, DIN), F32, kind="ExternalInput").ap()
    xtr = nc.dram_tensor("xtr", (P, 2, N), F32, kind="ExternalInput").ap()
    xloc = nc.dram_tensor("xloc", (P, NBLK, DIN), F32, kind="ExternalInput").ap()
    sup = nc.dram_tensor("supports", (K, NLOC, N), F16, kind="ExternalInput").ap()
    att = nc.dram_tensor("atten", (R, NLOC, N), F16, kind="ExternalInput").ap()
    wt = nc.dram_tensor("wt", (P, K, 2, DOUT), F32, kind="ExternalInput").ap()
    wl = nc.dram_tensor("wl", (K, R + 1, DOUT), F32, kind="ExternalInput").ap()
    wr = nc.dram_tensor("wr", (K, R + 1, DOUT), F32, kind="ExternalInput").ap()
    out = nc.dram_tensor("out", (DOUT, NLOC), F32, kind="ExternalOutput").ap()
    arts_d = nc.dram_tensor("arts_scratch", (K, 8, N), F16, kind="Internal").ap()

    with tile.TileContext(nc) as tc:
        build_kernel(tc, out, xg, xtr, xloc, sup, att, wt, wl, wr, arts_d)

    nc.compile()
    return nc


def build_kernel(tc, out, xg, xtr, xloc, sup, att, wt, wl, wr, arts_d):
    nc = tc.nc

    from contextlib import ExitStack

    ctx = ExitStack()
    with ctx:
        # ---------------- persistent pools ----------------
        p_const = ctx.enter_context(tc.tile_pool(name="const", bufs=1))
        p_xaug = ctx.enter_context(tc.tile_pool(name="xaug", bufs=1))
        p_w = ctx.enter_context(tc.tile_pool(name="wpool", bufs=1))
        p_small = ctx.enter_context(tc.tile_pool(name="small", bufs=1))
        p_gsb = ctx.enter_context(tc.tile_pool(name="gsb", bufs=NBLK * K))

        ps_s = ctx.enter_context(tc.tile_pool(name="ps_s", bufs=3, space="PSUM"))
        ps_g = ctx.enter_context(tc.tile_pool(name="ps_g", bufs=2, space="PSUM"))
        ps_o = ctx.enter_context(tc.tile_pool(name="ps_o", bufs=1, space="PSUM"))
        ps_m = ctx.enter_context(tc.tile_pool(name="ps_m", bufs=2, space="PSUM"))

        # ---------------- constants ----------------
        ident = p_const.tile([P, P], F16, tag="ident")
        make_identity(nc, ident[:])

        nshift = p_const.tile([P, 1], F32, tag="nshift")
        nc.vector.memset(nshift[:], -SHIFT)

        # ---------------- setup ----------------
        wt16 = p_w.tile([P, K, 2, DOUT], F16, tag="wt16")
        alr = p_small.tile([P, K, NBLK, 8], F32, tag="alr")
        xaug = p_xaug.tile([P, N // P, DIN + 1], F16, tag="xaug")

        # flat per-(k, c) AR broadcast tiles; the shared ring lets h=1
        # rebroadcasts start as soon as individual h=0 tiles retire
        p_ar = ctx.enter_context(tc.tile_pool(name="arbc", bufs=1))

        # main-loop mask pools, created early so the first block's mask DMAs
        # can be issued ahead of the whole setup DMA stream
        p_am = ctx.enter_context(tc.tile_pool(name="ampool", bufs=6))
        p_sm = ctx.enter_context(tc.tile_pool(name="smpool", bufs=3))
        pre_am = []
        for r in range(R):
            m16p = p_am.tile([P, H], F16, tag="am16", name=f"m16p{r}")
            nc.sync.dma_start(m16p[:], att[r, 0:P, 0:H])
            pre_am.append(m16p)
        pre_sm = []
        for k in range(K):
            smp = p_sm.tile([P, H], F16, tag="sm16", name=f"sm16p{k}")
            nc.sync.dma_start(smp[:], sup[k, 0:P, 0:H])
            pre_sm.append(smp)

        with tc.tile_pool(name="setup", bufs=2) as p_su:
            # weights fp16
            wts = p_su.tile([P, K, 2, DOUT], F32, tag="wts")
            nc.sync.dma_start(wts[:], wt[:])
            nc.vector.tensor_copy(wt16[:], wts[:])

            wlr32 = p_su.tile([R + 1, 2 * K * DOUT], F32, tag="wlr32")
            for k in range(K):
                nc.sync.dma_start(
                    wlr32[:, k * DOUT : (k + 1) * DOUT], wl[k, :, :]
                )
                nc.sync.dma_start(
                    wlr32[:, K * DOUT + k * DOUT : K * DOUT + (k + 1) * DOUT],
                    wr[k, :, :],
                )
            wlr16 = p_su.tile([R + 1, 2 * K * DOUT], F16, tag="wlr16")
            nc.vector.tensor_copy(wlr16[:], wlr32[:])

            # xlocT[ih] [128(i), 512(n)] fp16
            xls = p_su.tile([P, NBLK, DIN], F32, tag="xls")
            nc.sync.dma_start(xls[:], xloc[:])
            xl16 = p_su.tile([P, NBLK, DIN], F16, tag="xl16")
            nc.vector.tensor_copy(xl16[:], xls[:])
            xlts = []
            for ih in range(2):
                pst = ps_m.tile([P, NBLK * P], F32, tag="pst")
                for j in range(NBLK):
                    nc.tensor.matmul(
                        pst[:, P * j : P * (j + 1)],
                        xl16[:, j, P * ih : P * (ih + 1)],
                        ident[:],
                        start=True,
                        stop=True,
                        skip_group_check=True,
                    )
                xlt = p_su.tile([P, NBLK * P], F16, tag="xlt")
                nc.scalar.copy(xlt[:], pst[:])
                xlts.append(xlt)

            vcats = []
            for k in range(K):
                # W_t[k]^T  [128(d), 2(dh), 2(ih), 128(i)] fp16
                pst = ps_m.tile([P, 512], F32, tag="pst")
                for dh in range(2):
                    for ih in range(2):
                        nc.tensor.matmul(
                            pst[:, 256 * dh + 128 * ih : 256 * dh + 128 * ih + 128],
                            wt16[:, k, ih, P * dh : P * (dh + 1)],
                            ident[:],
                            start=True,
                            stop=True,
                            skip_group_check=True,
                        )
                wtT = p_su.tile([P, 2, 2, P], F16, tag="wtT")
                nc.vector.tensor_copy(wtT[:], pst[:])

                # [W_l[k] | W_r[k]]^T  [128(d), 2(dh), 2(lr), 4(r)] fp16
                pst2 = ps_m.tile([P, 16], F32, tag="pst")
                for dh in range(2):
                    for lr in range(2):
                        s0 = lr * K * DOUT + k * DOUT + P * dh
                        nc.tensor.matmul(
                            pst2[:, 8 * dh + 4 * lr : 8 * dh + 4 * lr + 4],
                            wlr16[:, s0 : s0 + P],
                            ident[0 : R + 1, 0 : R + 1],
                            start=True,
                            stop=True,
                            skip_group_check=True,
                        )
                wlrT = p_su.tile([P, 2, 2, R + 1], F16, tag="wlrT")
                nc.vector.tensor_copy(wlrT[:], pst2[:])

                # V = W_t[k] @ [W_l|W_r]^T : [256(i), 8(c)]
                psv = ps_m.tile([P, 16], F32, tag="pst")
                for ih in range(2):
                    for lr in range(2):
                        for dh in range(2):
                            nc.tensor.matmul(
                                psv[:, 8 * ih + 4 * lr : 8 * ih + 4 * lr + 4],
                                wtT[:, dh, ih, :],
                                wlrT[:, dh, lr, :],
                                start=(dh == 0),
                                stop=(dh == 1),
                                skip_group_check=True,
                            )
                vcat = p_su.tile([P, 2, 8], F16, tag="vcat", bufs=K)
                nc.vector.tensor_copy(vcat[:], psv[:])
                vcats.append(vcat)

                # al (+ ar for local rows): [128(n), 4(blk), 8(c)]
                psa = ps_m.tile([P, NBLK * 8], F32, tag="pst")
                for j in range(NBLK):
                    for ih in range(2):
                        nc.tensor.matmul(
                            psa[:, 8 * j : 8 * j + 8],
                            xlts[ih][:, P * j : P * (j + 1)],
                            vcat[:, ih, :],
                            start=(ih == 0),
                            stop=(ih == 1),
                            skip_group_check=True,
                        )
                nc.vector.tensor_copy(alr[:, k, :, :], psa[:])

            # xT[ih] [128(i), 4096(m)] fp16, pre-transposed on the host;
            # staged through half-width fp32 tiles to bound SBUF
            xts = []
            for ih in range(2):
                xt = p_su.tile([P, N], F16, tag="xt")
                for hh in range(2):
                    xt32 = p_su.tile([P, N // 2], F32, tag="xt32")
                    nc.sync.dma_start(
                        xt32[:], xtr[:, ih, hh * (N // 2) : (hh + 1) * (N // 2)]
                    )
                    nc.vector.tensor_copy(
                        xt[:, hh * (N // 2) : (hh + 1) * (N // 2)], xt32[:]
                    )
                xts.append(xt)

            for k in range(K):
                vcat = vcats[k]
                # arT = V^T @ xT : [8(c), 4096(m)] fp16
                for ch in range(N // 512):
                    psr = ps_m.tile([8, 512], F32, tag="pst")
                    for ih in range(2):
                        nc.tensor.matmul(
                            psr[:],
                            vcat[:, ih, :],
                            xts[ih][:, 512 * ch : 512 * (ch + 1)],
                            start=(ih == 0),
                            stop=(ih == 1),
                            skip_group_check=True,
                        )
                    ast = p_su.tile([8, 512], F16, tag="ast", bufs=3)
                    nc.scalar.copy(ast[:], psr[:])
                    nc.sync.dma_start(arts_d[k, :, 512 * ch : 512 * (ch + 1)], ast[:])

                # h=0 AR broadcast tiles: 0-stride DMA replicates the arts_d
                # row across all 128 partitions
                for c in range(NC):
                    arh = p_ar.tile([P, H], F16, tag="arbc", bufs=K * NC)
                    nc.sync.dma_start(
                        arh[:],
                        arts_d[k, 4 + c : 5 + c, 0:H].to_broadcast((P, H)),
                    )
                    _arh0[(k, c)] = arh

            # x staging: x_aug (fp16 x with ones column), contiguous DMAs.
            # Emitted after the arT/broadcast chain (the critical path to the
            # first main-loop iteration); its first consumer, the G matmul of
            # back() #1, runs ~4 iterations into the main loop.
            nc.vector.memset(xaug[:, :, DIN : DIN + 1], 1.0)
            for q in range(8):
                xs = p_su.tile([P, 4, DIN], F32, tag="xs")
                nc.sync.dma_start(xs[:], xg[:, 4 * q : 4 * q + 4, :])
                nc.vector.tensor_copy(xaug[:, 4 * q : 4 * q + 4, :DIN], xs[:])

        # ---------------- main-loop pools ----------------
        p_t = ctx.enter_context(tc.tile_pool(name="tmask", bufs=6))
        p_pt = ctx.enter_context(tc.tile_pool(name="ptile", bufs=2))
        p_epi = ctx.enter_context(tc.tile_pool(name="epi", bufs=1))

        # ---------------- back half of a tile (software-pipelined) ---------
        # Emitted one tile LATE so no engine's in-order stream stalls on the
        # transpose->exp latency of its own tile: while tile t's exp runs, the
        # engines are already executing tile t+1's front-half ops.
        def back(h, b, k, pt, sg):
            # zero invalid entries (S==0 exactly in fp32 PSUM -> sign 0):
            # pt = |pt * sign(S^T)|; the ACT Sign pass rides the same PSUM
            # read the exp does; the sign-bit clear is a paired bitwise AND
            nc.vector.tensor_tensor(pt[:], pt[:], sg[:], OP.mult)
            ptv = pt[:].bitcast(I32)
            nc.vector.tensor_scalar(ptv, ptv, 0x7FFF7FFF, None, OP.bitwise_and)

            gps = ps_g.tile([P, DIN + 1], F32, tag="gps")
            for g in range(NGRP):
                for j in range(GJ):
                    jj = h * NCH + GJ * g + j
                    ch = GJ * g + j
                    nc.tensor.matmul(
                        gps[:],
                        pt[:, P * ch : P * (ch + 1)],
                        xaug[:, jj, :],
                        start=(g == 0 and j == 0),
                        stop=(g == NGRP - 1 and j == GJ - 1),
                    )

            # accumulate G across halves in SBUF
            if h == 0:
                gsb = p_gsb.tile([P, DIN + 1], F16, tag="gsb")
                _gsb_store[(b, k)] = gsb
                nc.scalar.copy(gsb[:], gps[:])
                return
            gsb = _gsb_store[(b, k)]
            nc.vector.tensor_add(gsb[:], gsb[:], gps[:])

            # -------- normalize + project (only after half 1) ----
            zr = p_epi.tile([P, 1], F32, tag="zr")
            nc.vector.tensor_scalar_max(zr[:], gsb[:, DIN : DIN + 1], 1e-30)
            rz = p_epi.tile([P, 1], F32, tag="rz")
            nc.vector.reciprocal(rz[:], zr[:])
            gn = p_epi.tile([P, DIN], F16, tag="gn")
            nc.scalar.activation(gn[:], gsb[:, :DIN], AF.Copy, bias=0.0, scale=rz[:])

            gtp = ps_m.tile([P, DIN], F32, tag="pst")
            for ih in range(2):
                nc.tensor.matmul(
                    gtp[:, P * ih : P * (ih + 1)],
                    gn[:, P * ih : P * (ih + 1)],
                    ident[:],
                    start=True,
                    stop=True,
                    skip_group_check=True,
                )
            gt16 = p_epi.tile([P, DIN], F16, tag="gt16")
            nc.scalar.copy(gt16[:], gtp[:])

            fin = ps_o.tile([P, DOUT], F32, tag="ops")
            for dh in range(2):
                for ih in range(2):
                    nc.tensor.matmul(
                        fin[:, P * dh : P * (dh + 1)],
                        wt16[:, k, ih, P * dh : P * (dh + 1)],
                        gt16[:, P * ih : P * (ih + 1)],
                        start=(ih == 0),
                        stop=(ih == 1),
                        skip_group_check=True,
                    )
            if k == 0:
                osb = p_epi.tile([P, DOUT], F32, tag="osb", bufs=2)
                _out_store[b] = osb
                nc.vector.tensor_copy(osb[:], fin[:])
            else:
                osb = _out_store[b]
                nc.vector.tensor_add(osb[:], osb[:], fin[:])

            if k == K - 1:
                # -------- epilogue: ELU + store --------
                for dh in range(2):
                    xo = osb[:, P * dh : P * (dh + 1)]
                    ex = p_epi.tile([P, P], F32, tag="ex")
                    nc.scalar.activation(
                        ex[:], osb[:, P * dh : P * (dh + 1)], AF.Exp, bias=0.0, scale=1.0
                    )
                    em1 = p_epi.tile([P, P], F32, tag="em1")
                    # min(e^x - 1, 0): the negative branch of ELU
                    nc.vector.tensor_scalar(em1[:], ex[:], -1.0, 0.0, OP.add, OP.min)
                    rm = p_epi.tile([P, P], F32, tag="rm")
                    nc.vector.tensor_scalar_max(rm[:], xo, 0.0)
                    nc.vector.tensor_add(em1[:], em1[:], rm[:])
                    nc.sync.dma_start(
                        out[P * dh : P * (dh + 1), P * b : P * (b + 1)], em1[:]
                    )

        pending = []

        # ---------------- main loop ----------------
        for h in range(NH):
            # broadcast AR tiles for this half: AR[k][c] [128, H] fp16
            # per-k AR broadcast tiles [128, NC, H] fp16: doubling tree to 16
            # partitions, then 7 parallel DMAs (separate queues) to fill 128
            ar_t = {}
            for k in range(K):
                for c in range(NC):
                    if h == 0:
                        arh = _arh0[(k, c)]
                    else:
                        arh = p_ar.tile(
                            [P, H], F16, tag="arbc", bufs=K * NC
                        )
                        nc.sync.dma_start(
                            arh[:],
                            arts_d[
                                k, 4 + c : 5 + c, h * H : (h + 1) * H
                            ].to_broadcast((P, H)),
                        )
                    ar_t[(k, c)] = arh[:]

            for b in range(NBLK):
                # attention masks for (block, half): fp16 {0,1} straight from
                # HBM -- reused by all K hops. (h0, b0) was prefetched ahead
                # of setup.
                if h == 0 and b == 0:
                    am16 = pre_am
                else:
                    am16 = []
                    for r in range(R):
                        m16 = p_am.tile([P, H], F16, tag="am16", name=f"m16{r}")
                        nc.sync.dma_start(
                            m16[:],
                            att[r, P * b : P * (b + 1), h * H : (h + 1) * H],
                        )
                        am16.append(m16)

                for k in range(K):
                    if h == 0 and b == 0:
                        sm = pre_sm[k]
                    else:
                        sm = p_sm.tile([P, H], F16, tag="sm16")
                        nc.sync.dma_start(
                            sm[:],
                            sup[k, P * b : P * (b + 1), h * H : (h + 1) * H],
                        )

                    # T_c = (AR_c + al_c) * M_c, split per channel to balance
                    # DVE vs PE (the fused STT runs at 1x on DVE; TS is 4x,
                    # TT 2x; gpsimd contends with DVE for SBUF ports -- keep
                    # it idle):
                    #   c in PE_AL:  bc_c = AR_c * M_c  (one DVE TT-mult);
                    #                the al part rides the PE as a diag
                    #                matmul st=raw mask, mv=diag(al_c)
                    #   c not in PE_AL: TS-add then TT-mult on DVE
                    PE_AL = PE_AL_K[k]
                    masks = [am16[0], am16[1], am16[2], sm]
                    bcs = []
                    diags = []
                    for c in range(NC):
                        if c in PE_AL:
                            bc = p_t.tile(
                                [P, H], F16, tag="bc", bufs=8, name=f"bc{c}"
                            )
                            nc.vector.tensor_tensor(
                                bc[:], ar_t[(k, c)], masks[c][:], OP.mult
                            )
                            dg = p_t.tile(
                                [P, P], F16, tag="dg", bufs=6,
                                name=f"dg{c}",
                            )
                            nc.vector.tensor_scalar_mul(
                                dg[:], ident[:], alr[:, k, b, c : c + 1]
                            )
                            diags.append(dg)
                        else:
                            ba = p_t.tile(
                                [P, H], F16, tag="ba", bufs=3, name=f"ba{c}"
                            )
                            nc.vector.tensor_scalar_add(
                                ba[:], ar_t[(k, c)], alr[:, k, b, c : c + 1]
                            )
                            bc = p_t.tile(
                                [P, H], F16, tag="bc", bufs=8, name=f"bc{c}"
                            )
                            nc.vector.tensor_tensor(
                                bc[:], ba[:], masks[c][:], OP.mult
                            )
                            diags.append(None)
                        bcs.append(bc)

                    pt = p_pt.tile([P, H], F16, tag="ptile", bufs=5)
                    sg = p_pt.tile([P, H], F16, tag="sgtile", bufs=4)
                    for g in range(NGRP):
                        sT = ps_s.tile([P, GJ * P], F32, tag="sT")
                        for j in range(GJ):
                            ch = GJ * g + j
                            sl = sT[:, P * j : P * (j + 1)]
                            mms = []
                            for c in range(NC):
                                mms.append((bcs[c], ident))
                                if c in PE_AL:
                                    mms.append((masks[c], diags[c]))
                            for mi, (stat, mov) in enumerate(mms):
                                nc.tensor.matmul(
                                    sl,
                                    stat[:, P * ch : P * (ch + 1)],
                                    mov[:],
                                    start=(mi == 0),
                                    stop=(mi == len(mms) - 1),
                                    skip_group_check=True,
                                )
                        nc.scalar.activation(
                            pt[:, GJ * P * g : GJ * P * (g + 1)],
                            sT[:],
                            AF.Exp,
                            bias=nshift[:],
                            scale=1.0,
                        )
                        nc.scalar.activation(
                            sg[:, GJ * P * g : GJ * P * (g + 1)],
                            sT[:],
                            AF.Sign,
                            bias=0.0,
                            scale=1.0,
                        )

                    pending.append((h, b, k, pt, sg))
                    if len(pending) > 3:
                        back(*pending.pop(0))

        while pending:
            back(*pending.pop(0))


_gsb_store = {}
_out_store = {}
_arh0 = {}

_cache = threading.Lock()
_program = None


def _get_program():
    global _program
    with _cache:
        if _program is None:
            _gsb_store.clear()
            _out_store.clear()
            _arh0.clear()
            _program = build_program()
    return _program


def block_xg(x):
    """[N, DIN] -> partition-major [P, N//P, DIN] (contiguous device DMA)."""
    return np.ascontiguousarray(
        x.reshape(N // P, P, DIN).transpose(1, 0, 2), dtype=np.float32
    )


def block_xtr(x):
    """[N, DIN] -> x.T blocked [P, 2, N] (contiguous device DMA)."""
    return np.ascontiguousarray(
        x.T.reshape(2, P, N).transpose(1, 0, 2), dtype=np.float32
    )


def block_xloc(xl):
    return np.ascontiguousarray(
        xl.reshape(NBLK, P, DIN).transpose(1, 0, 2), dtype=np.float32
    )


def block_wt(W_t):
    """[K, DIN, DOUT] -> [P, K, 2, DOUT]."""
    return np.ascontiguousarray(
        W_t.reshape(K, 2, P, DOUT).transpose(2, 0, 1, 3), dtype=np.float32
    )


def kernel(x, supports, atten_supports, W_t, W_l, W_r):
    nc = _get_program()

    x = np.ascontiguousarray(x, dtype=np.float32)
    # masks are semantically boolean ((adj != 0) is the reference's first
    # op); ship them to the device as fp16 {0,1}
    supports = np.ascontiguousarray(supports != 0, dtype=np.float16)
    atten_supports = np.ascontiguousarray(atten_supports != 0, dtype=np.float16)
    xg_b = block_xg(x)
    xtr_b = block_xtr(x)
    wt_b = block_wt(np.asarray(W_t, dtype=np.float32))
    in_maps = []
    for i in range(NCORES):
        r0, r1 = i * NLOC, (i + 1) * NLOC
        in_maps.append(
            {
                "xg": xg_b,
                "xtr": xtr_b,
                "xloc": block_xloc(x[r0:r1]),
                "supports": np.ascontiguousarray(supports[:, r0:r1, :]),
                "atten": np.ascontiguousarray(atten_supports[:, r0:r1, :]),
                "wt": wt_b,
                "wl": np.ascontiguousarray(W_l, dtype=np.float32),
                "wr": np.ascontiguousarray(W_r, dtype=np.float32),
            }
        )

    from concourse import bass_utils

    res = bass_utils.run_bass_kernel_spmd(nc, in_maps, core_ids=list(range(NCORES)))
    out = np.concatenate(
        [np.asarray(res.results[i]["out"]).T for i in range(NCORES)], axis=0
    )
    return out.astype(np.float32)



# revision 52
# speedup vs baseline: 1.1701x; 1.1701x over previous
"""Trainium2 Bass kernel for nn_AttentionHeadCheb (gnn_message_passing).

8-core SPMD, row-sharded (destination-node dim). Each core owns 512 rows of
the [4096, 4096] score/prob matrices for all K=3 hops, streams its mask
shards from HBM once, and computes its 512 output rows. No collectives.

Masks are cast to fp16 {0,1} on the host (the reference's own first op is
`(adj != 0).astype(f32)`), halving HBM traffic vs int32; x is additionally
shipped pre-transposed (layout only) so no PE time is spent transposing it.

Engine cost model measured on HW (per [128, 2048] fp16 op): tensor_scalar
~660ns (4x dual-pump), tensor_tensor ~1140ns (2x), fused scalar_tensor_
tensor ~2350ns (1x -- avoid), PE matmul 56ns/128-col when ramped, gpsimd
~4000ns AND it contends with DVE for SBUF ports (keep it idle). The kernel
balances the score build across PE and DVE accordingly:

  setup:  x_aug  = cast16(x) with a ones column          (for P@x and rowsums)
          xT     = cast16(host-transposed x)             (DMA)
          V      = W_t[k] @ [W_l[k] | W_r[k]].T          (PE, tiny)
          al     = xloc @ V   (left coeffs, local rows)  (PE, tiny)
          arT    = V.T @ xT   (right coeffs, all cols)   (PE)
          AR_c   = broadcast arT rows across partitions  (0-stride DMA)
  main:   per channel c in {amask r=0,1,2 (PE_AL), smask_k}:
            PE_AL channels:  D_c = AR_c * M_c            (DVE TT-mult, 2x)
                             al part rides the PE:       S^T += M_c^T diag(al_c)
                             (stationary = raw mask, moving = diag(al_c);
                              diag built by a tiny DVE tensor_scalar_mul)
            else:            D_c = (AR_c + al_c) * M_c   (DVE TS-add + TT-mult)
          S^T    = sum_c transpose(D_c) (+ diag terms)   (PE transpose-accum,
                   7 matmuls per 128x128 chunk)
          p~T    = exp(S^T - 4), sg = sign(S^T)          (ACT, PSUM -> fp16)
          p~T    = |p~T * sg|                            (valid mask: DVE mult
                   + sign-bit clear; S==0 exactly <=> masked out, as reference)
          G_k    = p~T.T @ [x16 | 1]                     (PE, accumulated in PSUM)
  out:    G'_k   = G_k * (1/Z_k) ;  out^T = sum_k W_t[k].T @ G'_k^T   (PE)
          out    = elu(out^T)                            (ACT exp + DVE select)

Softmax needs no max-subtraction: |scores| <= ~9 for this distribution, the
-4 shift keeps exp() within fp16 range. Rows with no edges give Z=0 -> out 0,
matching the reference. The first block's mask DMAs are issued ahead of the
whole setup stream so the main loop can start as soon as the AR broadcast
tiles land. Steady state is PE-bound at ~99% tensor-engine occupancy.
"""

import threading

import numpy as np

import concourse.bass as bass
import concourse.tile as tile
from concourse import bacc, mybir
from concourse.masks import make_identity

N, DIN, DOUT, K, R = 4096, 256, 256, 3, 3
NCORES = 8
NLOC = N // NCORES  # 512 local rows per core
P = 128
NBLK = NLOC // P  # 4 row blocks
NH = 2  # column halves
H = N // NH  # 2048
NCH = H // P  # 16 column chunks per half
NGRP = 4  # psum S^T groups per half
GJ = NCH // NGRP  # 4 chunks per group
NC = 4  # mask channels: amask r=0..2 + smask
# channels whose al-add rides the PE as a diag matmul, per hop k;
# fractional balance: steady-state PE ran ~82% vs DVE ~74% with 3
PE_AL_K = {0: (0, 1, 2), 1: (0, 1, 2), 2: (0, 1, 2)}
SHIFT = 4.0  # exp shift for fp16 safety

F16 = mybir.dt.float16
F32 = mybir.dt.float32
I32 = mybir.dt.int32
I16 = mybir.dt.int16
AF = mybir.ActivationFunctionType
OP = mybir.AluOpType


def build_program():
    nc = bacc.Bacc(
        "TRN2",
        target_bir_lowering=False,
        debug=False,
        enable_asserts=False,
        num_devices=NCORES,
    )

    # xg/xloc/wt arrive pre-blocked from the host (partition-major) so the
    # staging DMAs read long contiguous per-partition lines
    xg = nc.dram_tensor("xg", (P, N // P, DIN), F32, kind="ExternalInput").ap()
    xtr = nc.dram_tensor("xtr", (P, 2, N), F32, kind="ExternalInput").ap()
    xloc = nc.dram_tensor("xloc", (P, NBLK, DIN), F32, kind="ExternalInput").ap()
    sup = nc.dram_tensor("supports", (K, NLOC, N), F16, kind="ExternalInput").ap()
    att = nc.dram_tensor("atten", (R, NLOC, N), F16, kind="ExternalInput").ap()
    wt = nc.dram_tensor("wt", (P, K, 2, DOUT), F32, kind="ExternalInput").ap()
    wl = nc.dram_tensor("wl", (K, R + 1, DOUT), F32, kind="ExternalInput").ap()
    wr = nc.dram_tensor("wr", (K, R + 1, DOUT), F32, kind="ExternalInput").ap()
    out = nc.dram_tensor("out", (DOUT, NLOC), F32, kind="ExternalOutput").ap()
    arts_d = nc.dram_tensor("arts_scratch", (K, 8, N), F16, kind="Internal").ap()

    with tile.TileContext(nc) as tc:
        build_kernel(tc, out, xg, xtr, xloc, sup, att, wt, wl, wr, arts_d)

    nc.compile()
    return nc


def build_kernel(tc, out, xg, xtr, xloc, sup, att, wt, wl, wr, arts_d):
    nc = tc.nc

    from contextlib import ExitStack

    ctx = ExitStack()
    with ctx:
        # ---------------- persistent pools ----------------
        p_const = ctx.enter_context(tc.tile_pool(name="const", bufs=1))
        p_xaug = ctx.enter_context(tc.tile_pool(name="xaug", bufs=1))
        p_w = ctx.enter_context(tc.tile_pool(name="wpool", bufs=1))
        p_small = ctx.enter_context(tc.tile_pool(name="small", bufs=1))
        p_gsb = ctx.enter_context(tc.tile_pool(name="gsb", bufs=NBLK * K))

        ps_s = ctx.enter_context(tc.tile_pool(name="ps_s", bufs=3, space="PSUM"))
        ps_g = ctx.enter_context(tc.tile_pool(name="ps_g", bufs=2, space="PSUM"))
        ps_o = ctx.enter_context(tc.tile_pool(name="ps_o", bufs=1, space="PSUM"))
        ps_m = ctx.enter_context(tc.tile_pool(name="ps_m", bufs=2, space="PSUM"))

        # ---------------- constants ----------------
        ident = p_const.tile([P, P], F16, tag="ident")
        make_identity(nc, ident[:])

        nshift = p_const.tile([P, 1], F32, tag="nshift")
        nc.vector.memset(nshift[:], -SHIFT)

        # ---------------- setup ----------------
        wt16 = p_w.tile([P, K, 2, DOUT], F16, tag="wt16")
        alr = p_small.tile([P, K, NBLK, 8], F32, tag="alr")
        xaug = p_xaug.tile([P, N // P, DIN + 1], F16, tag="xaug")

        # flat per-(k, c) AR broadcast tiles; the shared ring lets h=1
        # rebroadcasts start as soon as individual h=0 tiles retire
        p_ar = ctx.enter_context(tc.tile_pool(name="arbc", bufs=1))

        # main-loop mask pools, created early so the first block's mask DMAs
        # can be issued ahead of the whole setup DMA stream
        p_am = ctx.enter_context(tc.tile_pool(name="ampool", bufs=6))
        p_sm = ctx.enter_context(tc.tile_pool(name="smpool", bufs=3))
        pre_am = []
        for r in range(R):
            m16p = p_am.tile([P, H], F16, tag="am16", name=f"m16p{r}")
            nc.sync.dma_start(m16p[:], att[r, 0:P, 0:H])
            pre_am.append(m16p)
        pre_sm = []
        for k in range(K):
            smp = p_sm.tile([P, H], F16, tag="sm16", name=f"sm16p{k}")
            nc.sync.dma_start(smp[:], sup[k, 0:P, 0:H])
            pre_sm.append(smp)

        with tc.tile_pool(name="setup", bufs=2) as p_su:
            # weights fp16
            wts = p_su.tile([P, K, 2, DOUT], F32, tag="wts")
            nc.sync.dma_start(wts[:], wt[:])
            nc.vector.tensor_copy(wt16[:], wts[:])

            wlr32 = p_su.tile([R + 1, 2 * K * DOUT], F32, tag="wlr32")
            for k in range(K):
                nc.sync.dma_start(
                    wlr32[:, k * DOUT : (k + 1) * DOUT], wl[k, :, :]
                )
                nc.sync.dma_start(
                    wlr32[:, K * DOUT + k * DOUT : K * DOUT + (k + 1) * DOUT],
                    wr[k, :, :],
                )
            wlr16 = p_su.tile([R + 1, 2 * K * DOUT], F16, tag="wlr16")
            nc.vector.tensor_copy(wlr16[:], wlr32[:])

            # xlocT[ih] [128(i), 512(n)] fp16
            xls = p_su.tile([P, NBLK, DIN], F32, tag="xls")
            nc.sync.dma_start(xls[:], xloc[:])
            xl16 = p_su.tile([P, NBLK, DIN], F16, tag="xl16")
            nc.vector.tensor_copy(xl16[:], xls[:])
            xlts = []
            for ih in range(2):
                pst = ps_m.tile([P, NBLK * P], F32, tag="pst")
                for j in range(NBLK):
                    nc.tensor.matmul(
                        pst[:, P * j : P * (j + 1)],
                        xl16[:, j, P * ih : P * (ih + 1)],
                        ident[:],
                        start=True,
                        stop=True,
                        skip_group_check=True,
                    )
                xlt = p_su.tile([P, NBLK * P], F16, tag="xlt")
                nc.scalar.copy(xlt[:], pst[:])
                xlts.append(xlt)

            vcats = []
            for k in range(K):
                # W_t[k]^T  [128(d), 2(dh), 2(ih), 128(i)] fp16
                pst = ps_m.tile([P, 512], F32, tag="pst")
                for dh in range(2):
                    for ih in range(2):
                        nc.tensor.matmul(
                            pst[:, 256 * dh + 128 * ih : 256 * dh + 128 * ih + 128],
                            wt16[:, k, ih, P * dh : P * (dh + 1)],
                            ident[:],
                            start=True,
                            stop=True,
                            skip_group_check=True,
                        )
                wtT = p_su.tile([P, 2, 2, P], F16, tag="wtT")
                nc.vector.tensor_copy(wtT[:], pst[:])

                # [W_l[k] | W_r[k]]^T  [128(d), 2(dh), 2(lr), 4(r)] fp16
                pst2 = ps_m.tile([P, 16], F32, tag="pst")
                for dh in range(2):
                    for lr in range(2):
                        s0 = lr * K * DOUT + k * DOUT + P * dh
                        nc.tensor.matmul(
                            pst2[:, 8 * dh + 4 * lr : 8 * dh + 4 * lr + 4],
                            wlr16[:, s0 : s0 + P],
                            ident[0 : R + 1, 0 : R + 1],
                            start=True,
                            stop=True,
                            skip_group_check=True,
                        )
                wlrT = p_su.tile([P, 2, 2, R + 1], F16, tag="wlrT")
                nc.vector.tensor_copy(wlrT[:], pst2[:])

                # V = W_t[k] @ [W_l|W_r]^T : [256(i), 8(c)]
                psv = ps_m.tile([P, 16], F32, tag="pst")
                for ih in range(2):
                    for lr in range(2):
                        for dh in range(2):
                            nc.tensor.matmul(
                                psv[:, 8 * ih + 4 * lr : 8 * ih + 4 * lr + 4],
                                wtT[:, dh, ih, :],
                                wlrT[:, dh, lr, :],
                                start=(dh == 0),
                                stop=(dh == 1),
                                skip_group_check=True,
                            )
                vcat = p_su.tile([P, 2, 8], F16, tag="vcat", bufs=K)
                nc.vector.tensor_copy(vcat[:], psv[:])
                vcats.append(vcat)

                # al (+ ar for local rows): [128(n), 4(blk), 8(c)]
                psa = ps_m.tile([P, NBLK * 8], F32, tag="pst")
                for j in range(NBLK):
                    for ih in range(2):
                        nc.tensor.matmul(
                            psa[:, 8 * j : 8 * j + 8],
                            xlts[ih][:, P * j : P * (j + 1)],
                            vcat[:, ih, :],
                            start=(ih == 0),
                            stop=(ih == 1),
                            skip_group_check=True,
                        )
                nc.vector.tensor_copy(alr[:, k, :, :], psa[:])

            # x staging: x_aug (fp16 x with ones column), contiguous DMAs
            nc.vector.memset(xaug[:, :, DIN : DIN + 1], 1.0)
            for q in range(8):
                xs = p_su.tile([P, 4, DIN], F32, tag="xs")
                nc.sync.dma_start(xs[:], xg[:, 4 * q : 4 * q + 4, :])
                nc.vector.tensor_copy(xaug[:, 4 * q : 4 * q + 4, :DIN], xs[:])

            # xT[ih] [128(i), 4096(m)] fp16, pre-transposed on the host;
            # staged through half-width fp32 tiles to bound SBUF
            xts = []
            for ih in range(2):
                xt = p_su.tile([P, N], F16, tag="xt")
                for hh in range(2):
                    xt32 = p_su.tile([P, N // 2], F32, tag="xt32")
                    nc.sync.dma_start(
                        xt32[:], xtr[:, ih, hh * (N // 2) : (hh + 1) * (N // 2)]
                    )
                    nc.vector.tensor_copy(
                        xt[:, hh * (N // 2) : (hh + 1) * (N // 2)], xt32[:]
                    )
                xts.append(xt)

            for k in range(K):
                vcat = vcats[k]
                # arT = V^T @ xT : [8(c), 4096(m)] fp16
                for ch in range(N // 512):
                    psr = ps_m.tile([8, 512], F32, tag="pst")
                    for ih in range(2):
                        nc.tensor.matmul(
                            psr[:],
                            vcat[:, ih, :],
                            xts[ih][:, 512 * ch : 512 * (ch + 1)],
                            start=(ih == 0),
                            stop=(ih == 1),
                            skip_group_check=True,
                        )
                    ast = p_su.tile([8, 512], F16, tag="ast", bufs=3)
                    nc.scalar.copy(ast[:], psr[:])
                    nc.sync.dma_start(arts_d[k, :, 512 * ch : 512 * (ch + 1)], ast[:])

                # h=0 AR broadcast tiles: 0-stride DMA replicates the arts_d
                # row across all 128 partitions
                for c in range(NC):
                    arh = p_ar.tile([P, H], F16, tag="arbc", bufs=K * NC)
                    nc.sync.dma_start(
                        arh[:],
                        arts_d[k, 4 + c : 5 + c, 0:H].to_broadcast((P, H)),
                    )
                    _arh0[(k, c)] = arh

        # ---------------- main-loop pools ----------------
        p_t = ctx.enter_context(tc.tile_pool(name="tmask", bufs=6))
        p_pt = ctx.enter_context(tc.tile_pool(name="ptile", bufs=2))
        p_epi = ctx.enter_context(tc.tile_pool(name="epi", bufs=1))

        # ---------------- back half of a tile (software-pipelined) ---------
        # Emitted one tile LATE so no engine's in-order stream stalls on the
        # transpose->exp latency of its own tile: while tile t's exp runs, the
        # engines are already executing tile t+1's front-half ops.
        def back(h, b, k, pt, sg):
            # zero invalid entries (S==0 exactly in fp32 PSUM -> sign 0):
            # pt = |pt * sign(S^T)|; the ACT Sign pass rides the same PSUM
            # read the exp does; the sign-bit clear is a paired bitwise AND
            nc.vector.tensor_tensor(pt[:], pt[:], sg[:], OP.mult)
            ptv = pt[:].bitcast(I32)
            nc.vector.tensor_scalar(ptv, ptv, 0x7FFF7FFF, None, OP.bitwise_and)

            gps = ps_g.tile([P, DIN + 1], F32, tag="gps")
            for g in range(NGRP):
                for j in range(GJ):
                    jj = h * NCH + GJ * g + j
                    ch = GJ * g + j
                    nc.tensor.matmul(
                        gps[:],
                        pt[:, P * ch : P * (ch + 1)],
                        xaug[:, jj, :],
                        start=(g == 0 and j == 0),
                        stop=(g == NGRP - 1 and j == GJ - 1),
                    )

            # accumulate G across halves in SBUF
            if h == 0:
                gsb = p_gsb.tile([P, DIN + 1], F16, tag="gsb")
                _gsb_store[(b, k)] = gsb
                nc.scalar.copy(gsb[:], gps[:])
                return
            gsb = _gsb_store[(b, k)]
            nc.vector.tensor_add(gsb[:], gsb[:], gps[:])

            # -------- normalize + project (only after half 1) ----
            zr = p_epi.tile([P, 1], F32, tag="zr")
            nc.vector.tensor_scalar_max(zr[:], gsb[:, DIN : DIN + 1], 1e-30)
            rz = p_epi.tile([P, 1], F32, tag="rz")
            nc.vector.reciprocal(rz[:], zr[:])
            gn = p_epi.tile([P, DIN], F16, tag="gn")
            nc.scalar.activation(gn[:], gsb[:, :DIN], AF.Copy, bias=0.0, scale=rz[:])

            gtp = ps_m.tile([P, DIN], F32, tag="pst")
            for ih in range(2):
                nc.tensor.matmul(
                    gtp[:, P * ih : P * (ih + 1)],
                    gn[:, P * ih : P * (ih + 1)],
                    ident[:],
                    start=True,
                    stop=True,
                    skip_group_check=True,
                )
            gt16 = p_epi.tile([P, DIN], F16, tag="gt16")
            nc.scalar.copy(gt16[:], gtp[:])

            fin = ps_o.tile([P, DOUT], F32, tag="ops")
            for dh in range(2):
                for ih in range(2):
                    nc.tensor.matmul(
                        fin[:, P * dh : P * (dh + 1)],
                        wt16[:, k, ih, P * dh : P * (dh + 1)],
                        gt16[:, P * ih : P * (ih + 1)],
                        start=(ih == 0),
                        stop=(ih == 1),
                        skip_group_check=True,
                    )
            if k == 0:
                osb = p_epi.tile([P, DOUT], F32, tag="osb", bufs=2)
                _out_store[b] = osb
                nc.vector.tensor_copy(osb[:], fin[:])
            else:
                osb = _out_store[b]
                nc.vector.tensor_add(osb[:], osb[:], fin[:])

            if k == K - 1:
                # -------- epilogue: ELU + store --------
                for dh in range(2):
                    xo = osb[:, P * dh : P * (dh + 1)]
                    ex = p_epi.tile([P, P], F32, tag="ex")
                    nc.scalar.activation(
                        ex[:], osb[:, P * dh : P * (dh + 1)], AF.Exp, bias=0.0, scale=1.0
                    )
                    em1 = p_epi.tile([P, P], F32, tag="em1")
                    # min(e^x - 1, 0): the negative branch of ELU
                    nc.vector.tensor_scalar(em1[:], ex[:], -1.0, 0.0, OP.add, OP.min)
                    rm = p_epi.tile([P, P], F32, tag="rm")
                    nc.vector.tensor_scalar_max(rm[:], xo, 0.0)
                    nc.vector.tensor_add(em1[:], em1[:], rm[:])
                    nc.sync.dma_start(
                        out[P * dh : P * (dh + 1), P * b : P * (b + 1)], em1[:]
                    )

        pending = []

        # ---------------- main loop ----------------
        for h in range(NH):
            # broadcast AR tiles for this half: AR[k][c] [128, H] fp16
            # per-k AR broadcast tiles [128, NC, H] fp16: doubling tree to 16
            # partitions, then 7 parallel DMAs (separate queues) to fill 128
            ar_t = {}
            for k in range(K):
                for c in range(NC):
                    if h == 0:
                        arh = _arh0[(k, c)]
                    else:
                        arh = p_ar.tile(
                            [P, H], F16, tag="arbc", bufs=K * NC
                        )
                        nc.sync.dma_start(
                            arh[:],
                            arts_d[
                                k, 4 + c : 5 + c, h * H : (h + 1) * H
                            ].to_broadcast((P, H)),
                        )
                    ar_t[(k, c)] = arh[:]

            for b in range(NBLK):
                # attention masks for (block, half): fp16 {0,1} straight from
                # HBM -- reused by all K hops. (h0, b0) was prefetched ahead
                # of setup.
                if h == 0 and b == 0:
                    am16 = pre_am
                else:
                    am16 = []
                    for r in range(R):
                        m16 = p_am.tile([P, H], F16, tag="am16", name=f"m16{r}")
                        nc.sync.dma_start(
                            m16[:],
                            att[r, P * b : P * (b + 1), h * H : (h + 1) * H],
                        )
                        am16.append(m16)

                for k in range(K):
                    if h == 0 and b == 0:
                        sm = pre_sm[k]
                    else:
                        sm = p_sm.tile([P, H], F16, tag="sm16")
                        nc.sync.dma_start(
                            sm[:],
                            sup[k, P * b : P * (b + 1), h * H : (h + 1) * H],
                        )

                    # T_c = (AR_c + al_c) * M_c, split per channel to balance
                    # DVE vs PE (the fused STT runs at 1x on DVE; TS is 4x,
                    # TT 2x; gpsimd contends with DVE for SBUF ports -- keep
                    # it idle):
                    #   c in PE_AL:  bc_c = AR_c * M_c  (one DVE TT-mult);
                    #                the al part rides the PE as a diag
                    #                matmul st=raw mask, mv=diag(al_c)
                    #   c not in PE_AL: TS-add then TT-mult on DVE
                    PE_AL = PE_AL_K[k]
                    masks = [am16[0], am16[1], am16[2], sm]
                    bcs = []
                    diags = []
                    for c in range(NC):
                        if c in PE_AL:
                            bc = p_t.tile(
                                [P, H], F16, tag="bc", bufs=8, name=f"bc{c}"
                            )
                            nc.vector.tensor_tensor(
                                bc[:], ar_t[(k, c)], masks[c][:], OP.mult
                            )
                            dg = p_t.tile(
                                [P, P], F16, tag="dg", bufs=6,
                                name=f"dg{c}",
                            )
                            nc.vector.tensor_scalar_mul(
                                dg[:], ident[:], alr[:, k, b, c : c + 1]
                            )
                            diags.append(dg)
                        else:
                            ba = p_t.tile(
                                [P, H], F16, tag="ba", bufs=3, name=f"ba{c}"
                            )
                            nc.vector.tensor_scalar_add(
                                ba[:], ar_t[(k, c)], alr[:, k, b, c : c + 1]
                            )
                            bc = p_t.tile(
                                [P, H], F16, tag="bc", bufs=8, name=f"bc{c}"
                            )
                            nc.vector.tensor_tensor(
                                bc[:], ba[:], masks[c][:], OP.mult
                            )
                            diags.append(None)
                        bcs.append(bc)

                    pt = p_pt.tile([P, H], F16, tag="ptile", bufs=5)
                    sg = p_pt.tile([P, H], F16, tag="sgtile", bufs=4)
                    for g in range(NGRP):
                        sT = ps_s.tile([P, GJ * P], F32, tag="sT")
                        for j in range(GJ):
                            ch = GJ * g + j
                            sl = sT[:, P * j : P * (j + 1)]
                            mms = []
                            for c in range(NC):
                                mms.append((bcs[c], ident))
                                if c in PE_AL:
                                    mms.append((masks[c], diags[c]))
                            for mi, (stat, mov) in enumerate(mms):
                                nc.tensor.matmul(
                                    sl,
                                    stat[:, P * ch : P * (ch + 1)],
                                    mov[:],
                                    start=(mi == 0),
                                    stop=(mi == len(mms) - 1),
                                    skip_group_check=True,
                                )
                        nc.scalar.activation(
                            pt[:, GJ * P * g : GJ * P * (g + 1)],
                            sT[:],
                            AF.Exp,
                            bias=nshift[:],
                            scale=1.0,
                        )
                        nc.scalar.activation(
                            sg[:, GJ * P * g : GJ * P * (g + 1)],
                            sT[:],
                            AF.Sign,
                            bias=0.0,
                            scale=1.0,
                        )

                    pending.append((h, b, k, pt, sg))
                    if len(pending) > 3:
                        back(*pending.pop(0))

        while pending:
            back(*pending.pop(0))


_gsb_store = {}
_out_store = {}
_arh0 = {}

_cache = threading.Lock()
_program = None


def _get_program():
    global _program
    with _cache:
        if _program is None:
            _gsb_store.clear()
            _out_store.clear()
            _arh0.clear()
            _program = build_program()
    return _program


def block_xg(x):
    """[N, DIN] -> partition-major [P, N//P, DIN] (contiguous device DMA)."""
    return np.ascontiguousarray(
        x.reshape(N // P, P, DIN).transpose(1, 0, 2), dtype=np.float32
    )


def block_xtr(x):
    """[N, DIN] -> x.T blocked [P, 2, N] (contiguous device DMA)."""
    return np.ascontiguousarray(
        x.T.reshape(2, P, N).transpose(1, 0, 2), dtype=np.float32
    )


def block_xloc(xl):
    return np.ascontiguousarray(
        xl.reshape(NBLK, P, DIN).transpose(1, 0, 2), dtype=np.float32
    )


def block_wt(W_t):
    """[K, DIN, DOUT] -> [P, K, 2, DOUT]."""
    return np.ascontiguousarray(
        W_t.reshape(K, 2, P, DOUT).transpose(2, 0, 1, 3), dtype=np.float32
    )


def kernel(x, supports, atten_supports, W_t, W_l, W_r):
    nc = _get_program()

    x = np.ascontiguousarray(x, dtype=np.float32)
    # masks are semantically boolean ((adj != 0) is the reference's first
    # op); ship them to the device as fp16 {0,1}
    supports = np.ascontiguousarray(supports != 0, dtype=np.float16)
    atten_supports = np.ascontiguousarray(atten_supports != 0, dtype=np.float16)
    xg_b = block_xg(x)
    xtr_b = block_xtr(x)
    wt_b = block_wt(np.asarray(W_t, dtype=np.float32))
    in_maps = []
    for i in range(NCORES):
        r0, r1 = i * NLOC, (i + 1) * NLOC
        in_maps.append(
            {
                "xg": xg_b,
                "xtr": xtr_b,
                "xloc": block_xloc(x[r0:r1]),
                "supports": np.ascontiguousarray(supports[:, r0:r1, :]),
                "atten": np.ascontiguousarray(atten_supports[:, r0:r1, :]),
                "wt": wt_b,
                "wl": np.ascontiguousarray(W_l, dtype=np.float32),
                "wr": np.ascontiguousarray(W_r, dtype=np.float32),
            }
        )

    from concourse import bass_utils

    res = bass_utils.run_bass_kernel_spmd(nc, in_maps, core_ids=list(range(NCORES)))
    out = np.concatenate(
        [np.asarray(res.results[i]["out"]).T for i in range(NCORES)], axis=0
    )
    return out.astype(np.float32)

